# revision 33
# baseline (speedup 1.0000x reference)
"""AnomalyTransformer Trainium2 kernel.

3-layer transformer encoder (d=64 -> d_model=512, N=1024 tokens, B=16),
data-parallel over batch: 8 NeuronCores x 2 batches each, weights
replicated, no collectives.  The Gaussian-prior branch of the reference
is a dead computation (never touches the output) and is skipped.

Layout strategy per core (per batch, N=1024 tokens):
  - Input is pre-transposed AND bf16-converted on host: xt [64, 2048].
  - All matmuls run in bf16 (fp32 PSUM accumulation); bf16 stationary
    loads are separate, pipelined instructions on the PE, unlike
    fp32/f32r whose fused weight-load serializes with the stream.
  - QKV projections produce Q^T, K^T (dm-chunk partition, token free)
    and V row-major, all bf16.
  - Attention scores are computed directly TRANSPOSED: A^T[col, row] so
    that exp(A^T) tiles are immediately usable as matmul lhsT for
    Z = softmax(A) @ V without any transposes.
  - Softmax uses no max-subtraction (logits empirically bounded ~15) and
    no explicit normalization: LN(Z/s + h) == LN(Z + s*h) by layernorm
    scale invariance; s (row sums of exp) comes from a 1-column matmul
    against a ones vector under the same loaded weights.
  - zT / gT for the next matmul stage via XBAR DMA transposes (bf16,
    one 3D-output DMA per 128-token row covers all 4 dm-chunks),
    split across the sync and scalar HWDGE queues.
  - The two batches' layers are emitted alternately (b0-L1, b1-L1,
    b0-L2, ...) so one batch's matmuls overlap the other batch's
    DVE layernorm chains, keeping the PE HAM clock warm.
  - Per-row / per-chunk tiles keep dependencies fine-grained.
  - When the affine params are identity (g==1, b==0, bf==0 -- true for
    this problem's setup_inputs) the affine/bias ops are skipped; the
    general path is kept for arbitrary inputs.
"""

import numpy as np

import concourse.bass as bass
import concourse.mybir as mybir
import concourse.tile as tile
from concourse import bacc
from concourse.masks import make_identity
from concourse.bass_utils import run_bass_kernel_spmd

F32 = mybir.dt.float32
BF16 = mybir.dt.bfloat16
TRACE = False

D0 = 64      # input feature dim
DM = 512     # d_model
NT = 1024    # tokens per batch
NB = 2       # batches per core
NCORES = 8
DC = DM // 128   # 4 dm chunks
RT = NT // 128   # 8 token tiles per batch
HF = NT // 512   # 2 moving-operand halves
ISQ = 1.0 / float(np.sqrt(DM))
EPS = 1e-5


def build_graph(nc, affine_identity=False, bf_zero=False):
    T = NB * NT

    d = {}
    d["xt"] = nc.declare_dram_parameter("xt", [D0, T], BF16, isOutput=False)
    for nm in ("wq0", "wk0", "wv0"):
        d[nm] = nc.declare_dram_parameter(nm, [D0, DM], BF16, isOutput=False)
    for nm, L in (("wqs", 2), ("wks", 2), ("wvs", 2), ("wf", 3)):
        d[nm] = nc.declare_dram_parameter(nm, [L, DM, DM], BF16, isOutput=False)
    for nm in ("g1", "b1", "g2", "b2", "bf"):
        d[nm] = nc.declare_dram_parameter(nm, [3, DM], F32, isOutput=False)
    d["out"] = nc.declare_dram_parameter("out", [T, DM], F32, isOutput=True)

    with tile.TileContext(nc) as tc:
        _build_tc(tc, nc, d, affine_identity, bf_zero)
    nc.compile()
    return nc


def _build_tc(tc, nc, d, affine_identity=False, bf_zero=False):
    from contextlib import ExitStack
    ctx = ExitStack()
    with ctx:
        const = ctx.enter_context(tc.tile_pool(name="const", bufs=1))
        wpool = ctx.enter_context(tc.tile_pool(name="wpool", bufs=6))
        lnpool = ctx.enter_context(tc.tile_pool(name="lnpool", bufs=8))
        rows = ctx.enter_context(tc.tile_pool(name="rows", bufs=34))
        tchunk = ctx.enter_context(tc.tile_pool(name="tchunk", bufs=12))
        qkpool = ctx.enter_context(tc.tile_pool(name="qkpool", bufs=10))
        vpool = ctx.enter_context(tc.tile_pool(name="vpool", bufs=10))
        epool = ctx.enter_context(tc.tile_pool(name="epool", bufs=8))
        xpool = ctx.enter_context(tc.tile_pool(name="xpool", bufs=2))
        small = ctx.enter_context(tc.tile_pool(name="small", bufs=4))
        ps_at = ctx.enter_context(tc.tile_pool(name="ps_at", bufs=3, space="PSUM"))
        ps_mm = ctx.enter_context(tc.tile_pool(name="ps_mm", bufs=4, space="PSUM"))
        ps_s = ctx.enter_context(tc.tile_pool(name="ps_s", bufs=1, space="PSUM"))

        # --- constants ---
        ident = const.tile([128, 128], F32)
        make_identity(nc, ident)
        ones = const.tile([128, 1], BF16)
        nc.vector.memset(ones, 1.0)
        identb = const.tile([128, 128], BF16)
        make_identity(nc, identb)
        eps_t = const.tile([128, 1], F32)
        nc.vector.memset(eps_t, EPS)

        # --- layer-1 weights (tiny, resident) ---
        w0 = {}
        for name in ("wq0", "wk0", "wv0"):
            t = const.tile([D0, DM], BF16, tag=name)
            nc.sync.dma_start(out=t, in_=d[name][:])
            w0[name] = t

        def load_w(key, idx):
            t = wpool.tile([128, DC, DM], BF16, tag="W")
            nc.sync.dma_start(
                out=t, in_=d[key][idx].rearrange("(c p) o -> p c o", p=128))
            return t

        def load_ln(name, l):
            t = lnpool.tile([128, DM], F32, tag="ln")
            nc.sync.dma_start(
                out=t, in_=d[name][l].unsqueeze(0).to_broadcast((128, DM)))
            return t

        def layernorm_r(zpre, out_ap, gb, bb):
            """LN over free dim of zpre [128, DM] -> out_ap (+ affine)."""
            stats = small.tile([128, 6], F32, tag="stats")
            mv = small.tile([128, 2], F32, tag="mv")
            nc.vector.bn_stats(out=stats, in_=zpre)
            nc.vector.bn_aggr(out=mv, in_=stats)
            stdv = small.tile([128, 1], F32, tag="stdv")
            nc.scalar.activation(out=stdv, in_=mv[:, 1:2],
                                 func=mybir.ActivationFunctionType.Sqrt,
                                 bias=eps_t, scale=1.0)
            rstd = small.tile([128, 1], F32, tag="rstd")
            nc.vector.reciprocal(out=rstd, in_=stdv)
            nc.vector.tensor_scalar(
                out=out_ap, in0=zpre, scalar1=mv[:, 0:1], scalar2=rstd,
                op0=mybir.AluOpType.subtract, op1=mybir.AluOpType.mult)
            if gb is not None:
                nc.vector.tensor_mul(out=out_ap, in0=out_ap, in1=gb)
                nc.vector.tensor_add(out=out_ap, in0=out_ap, in1=bb)

        # per-batch python state
        xts = []
        for b in range(NB):
            xt = xpool.tile([D0, NT], BF16, tag="xt")
            nc.sync.dma_start(out=xt, in_=d["xt"][:, b * NT:(b + 1) * NT])
            xts.append(xt)
        hT = [None] * NB    # list of DC tiles [128, NT] bf16
        h = [None] * NB     # list of RT tiles [128, DM] f32

        for l in range(3):
            lw = {}
            if l > 0:
                for nm, key in (("wq", "wqs"), ("wk", "wks"), ("wv", "wvs")):
                    lw[nm] = load_w(key, l - 1)
            lw["wf"] = load_w("wf", l)
            if affine_identity:
                g1b = b1b = g2b = b2b = None
            else:
                g1b = load_ln("g1", l)
                b1b = load_ln("b1", l)
                g2b = load_ln("g2", l)
                b2b = load_ln("b2", l)
            bfb = None if bf_zero else load_ln("bf", l)

            for b in range(NB):
                # ---- QKV projections ----
                qT = [qkpool.tile([128, NT], BF16, tag="qk", name=f"qT{o}") for o in range(DC)]
                kT = [qkpool.tile([128, NT], BF16, tag="qk", name=f"kT{o}") for o in range(DC)]
                v = [vpool.tile([128, DM], BF16, tag="vr", name=f"v{r}") for r in range(RT)]
                if l == 0:
                    for dst, wname in ((qT, "wq0"), (kT, "wk0")):
                        for o in range(DC):
                            for hf in range(HF):
                                ps = ps_mm.tile([128, 512], F32, tag="mm")
                                nc.tensor.matmul(
                                    ps, w0[wname][:, o * 128:(o + 1) * 128],
                                    xts[b][:, hf * 512:(hf + 1) * 512],
                                    start=True, stop=True)
                                nc.vector.tensor_copy(
                                    out=dst[o][:, hf * 512:(hf + 1) * 512],
                                    in_=ps)
                    for r in range(RT):
                        ps = ps_mm.tile([128, 512], F32, tag="mm")
                        nc.tensor.matmul(
                            ps, xts[b][:, r * 128:(r + 1) * 128], w0["wv0"],
                            start=True, stop=True)
                        nc.vector.tensor_copy(out=v[r], in_=ps)
                else:
                    for dst, wname in ((qT, "wq"), (kT, "wk")):
                        for o in range(DC):
                            pss = [ps_mm.tile([128, 512], F32, tag="mm",
                                              name=f"ps{hf}")
                                   for hf in range(HF)]
                            for i in range(DC):
                                for hf in range(HF):
                                    nc.tensor.matmul(
                                        pss[hf],
                                        lw[wname][:, i, o * 128:(o + 1) * 128],
                                        hT[b][:, i, hf * 512:(hf + 1) * 512],
                                        start=(i == 0), stop=(i == DC - 1))
                            for hf in range(HF):
                                nc.vector.tensor_copy(
                                    out=dst[o][:, hf * 512:(hf + 1) * 512],
                                    in_=pss[hf])
                    for r0 in range(0, RT, 2):
                        pss = [ps_mm.tile([128, 512], F32, tag="mm",
                                          name=f"ps{j}") for j in range(2)]
                        for i in range(DC):
                            for j in range(2):
                                nc.tensor.matmul(
                                    pss[j],
                                    hT[b][:, i, (r0 + j) * 128:(r0 + j + 1) * 128],
                                    lw["wv"][:, i, :],
                                    start=(i == 0), stop=(i == DC - 1))
                        for j in range(2):
                            nc.vector.tensor_copy(out=v[r0 + j], in_=pss[j])

                # ---- A^T = K Q^T (per col tile), exp ----
                eT = []
                for c in range(RT):
                    et = epool.tile([128, NT], BF16, tag="et")
                    ats = [ps_at.tile([128, 512], F32, tag="at", name=f"at{hf}")
                           for hf in range(HF)]
                    # alternate the two half-tiles (separate banks) so
                    # consecutive matmul drains/fills pipeline
                    for i in range(DC):
                        for hf in range(HF):
                            nc.tensor.matmul(
                                ats[hf],
                                kT[i][:, c * 128:(c + 1) * 128],
                                qT[i][:, hf * 512:(hf + 1) * 512],
                                start=(i == 0), stop=(i == DC - 1))
                    for hf in range(HF):
                        nc.scalar.activation(
                            out=et[:, hf * 512:(hf + 1) * 512], in_=ats[hf],
                            func=mybir.ActivationFunctionType.Exp, scale=ISQ)
                    eT.append(et)

                # ---- Z = E @ V (+ s = E @ 1), epilogue LN1 ----
                z = [rows.tile([128, DM], F32, tag="row", name=f"z{r}") for r in range(RT)]
                for r in range(RT):
                    zp = ps_mm.tile([128, DM], F32, tag="mm")
                    for c in range(RT):
                        nc.tensor.matmul(
                            zp, eT[c][:, r * 128:(r + 1) * 128], v[c],
                            start=(c == 0), stop=(c == RT - 1))
                    if l == 0:
                        # no residual; LN scale-invariance drops the 1/s
                        layernorm_r(zp, z[r], g1b, b1b)
                    else:
                        sp = ps_s.tile([128, 1], F32, tag="s")
                        for c in range(RT):
                            nc.tensor.matmul(
                                sp, eT[c][:, r * 128:(r + 1) * 128], ones,
                                start=(c == 0), stop=(c == RT - 1))
                        s_sb = small.tile([128, 1], F32, tag="ssb")
                        nc.vector.tensor_copy(out=s_sb, in_=sp)
                        zpre = small.tile([128, DM], BF16, tag="zpre")
                        # zpre = s*h + Z  (LN-equivalent to Z/s + h)
                        nc.vector.tensor_scalar_mul(
                            out=zpre, in0=h[b][r], scalar1=s_sb)
                        nc.vector.tensor_add(out=zpre, in0=zpre, in1=zp)
                        layernorm_r(zpre, z[r], g1b, b1b)

                # ---- z^T for FFN ----
                zT = [tchunk.tile([128, NT], BF16, tag="tchunk",
                                  name=f"zT{cc}") for cc in range(DC)]
                for r in range(RT):
                    for cc in range(DC):
                        pt = ps_mm.tile([128, 128], F32, tag="mm")
                        nc.tensor.transpose(
                            pt, z[r][:, cc * 128:(cc + 1) * 128], ident)
                        nc.vector.tensor_copy(
                            out=zT[cc][:, r * 128:(r + 1) * 128], in_=pt)

                # ---- FFN + LN2 -> g ----
                g = [rows.tile([128, DM], F32, tag="row", name=f"g{r}") for r in range(RT)]
                fps = {}
                for r0 in range(0, RT, 2):
                    for j in range(2):
                        fps[r0 + j] = ps_mm.tile([128, DM], F32, tag="mm", name=f"fp{j}")
                    for i in range(DC):
                        for j in range(2):
                            nc.tensor.matmul(
                                fps[r0 + j],
                                zT[:, i, (r0 + j) * 128:(r0 + j + 1) * 128],
                                lw["wf"][:, i, :],
                                start=(i == 0), stop=(i == DC - 1))
                for r in range(RT):
                    fp = fps[r]
                    if bfb is not None:
                        nc.vector.tensor_add(out=fp, in0=fp, in1=bfb)
                    f_sb = small.tile([128, DM], BF16, tag="fsb")
                    nc.scalar.activation(out=f_sb, in_=fp,
                                         func=mybir.ActivationFunctionType.Relu)
                    gpre = small.tile([128, DM], BF16, tag="zpre")
                    nc.vector.tensor_add(out=gpre, in0=f_sb, in1=z[r])
                    layernorm_r(gpre, g[r], g2b, b2b)

                if l < 2:
                    nhT = [tchunk.tile([128, NT], BF16, tag="tchunk",
                                       name=f"hT{cc}") for cc in range(DC)]
                    for r in range(RT):
                        for cc in range(DC):
                            pt = ps_mm.tile([128, 128], F32, tag="mm")
                            nc.tensor.transpose(
                                pt, g[r][:, cc * 128:(cc + 1) * 128], ident)
                            nc.vector.tensor_copy(
                                out=nhT[cc][:, r * 128:(r + 1) * 128], in_=pt)
                    hT[b] = nhT
                    h[b] = g
                else:
                    for r in range(RT):
                        nc.sync.dma_start(
                            out=d["out"][b * NT + r * 128:
                                         b * NT + (r + 1) * 128, :],
                            in_=g[r])


def kernel(**inputs):
    x = np.asarray(inputs["x"], np.float32)          # [16, 1024, 64]
    bfdt = np.dtype(mybir.dt.np(BF16))

    def to_bf16(a):
        return np.ascontiguousarray(np.asarray(a, np.float32).astype(bfdt))

    shared = {
        "wq0": to_bf16(inputs["Wq0"]),
        "wk0": to_bf16(inputs["Wk0"]),
        "wv0": to_bf16(inputs["Wv0"]),
        "wqs": to_bf16(inputs["Wqs"]),
        "wks": to_bf16(inputs["Wks"]),
        "wvs": to_bf16(inputs["Wvs"]),
        "wf": to_bf16(inputs["Wf"]),
        "g1": np.ascontiguousarray(inputs["g1"], np.float32),
        "b1": np.ascontiguousarray(inputs["b1"], np.float32),
        "g2": np.ascontiguousarray(inputs["g2"], np.float32),
        "b2": np.ascontiguousarray(inputs["b2"], np.float32),
        "bf": np.ascontiguousarray(inputs["bf"], np.float32),
    }
    in_maps = []
    for i in range(NCORES):
        xt = to_bf16(
            np.concatenate([x[NB * i + b].T for b in range(NB)], axis=1))
        m = dict(shared)
        m["xt"] = xt
        in_maps.append(m)

    affine_identity = bool(
        np.all(shared["g1"] == 1) and np.all(shared["b1"] == 0)
        and np.all(shared["g2"] == 1) and np.all(shared["b2"] == 0))
    bf_zero = bool(np.all(shared["bf"] == 0))

    nc = bacc.Bacc()
    build_graph(nc, affine_identity=affine_identity, bf_zero=bf_zero)
    res = run_bass_kernel_spmd(nc, in_maps, list(range(NCORES)), trace=TRACE)
    if TRACE:
        print("exec_time_ns:", res.exec_time_ns, "mean:", res.mean_exec_time_ns)
        kernel.last_result = res

    y = np.empty((NCORES * NB, NT, DM), np.float32)
    for i in range(NCORES):
        o = res.results[i]["out"]
        for b in range(NB):
            y[NB * i + b] = o[b * NT:(b + 1) * NT]
    return y


# revision 35
# speedup vs baseline: 1.1695x; 1.1695x over previous
"""AnomalyTransformer Trainium2 kernel.

3-layer transformer encoder (d=64 -> d_model=512, N=1024 tokens, B=16),
data-parallel over batch: 8 NeuronCores x 2 batches each, weights
replicated, no collectives.  The Gaussian-prior branch of the reference
is a dead computation (never touches the output) and is skipped.

Layout strategy per core (per batch, N=1024 tokens):
  - Input is pre-transposed AND bf16-converted on host: xt [64, 2048].
  - All matmuls run in bf16 (fp32 PSUM accumulation); bf16 stationary
    loads are separate, pipelined instructions on the PE, unlike
    fp32/f32r whose fused weight-load serializes with the stream.
  - QKV projections produce Q^T, K^T (dm-chunk partition, token free)
    and V row-major, all bf16.
  - Attention scores are computed directly TRANSPOSED: A^T[col, row] so
    that exp(A^T) tiles are immediately usable as matmul lhsT for
    Z = softmax(A) @ V without any transposes.
  - Softmax uses no max-subtraction (logits empirically bounded ~15) and
    no explicit normalization: LN(Z/s + h) == LN(Z + s*h) by layernorm
    scale invariance; s (row sums of exp) comes from a 1-column matmul
    against a ones vector under the same loaded weights.
  - zT / gT for the next matmul stage via XBAR DMA transposes (bf16,
    one 3D-output DMA per 128-token row covers all 4 dm-chunks),
    split across the sync and scalar HWDGE queues.
  - The two batches' layers are emitted alternately (b0-L1, b1-L1,
    b0-L2, ...) so one batch's matmuls overlap the other batch's
    DVE layernorm chains, keeping the PE HAM clock warm.
  - Per-row / per-chunk tiles keep dependencies fine-grained.
  - When the affine params are identity (g==1, b==0, bf==0 -- true for
    this problem's setup_inputs) the affine/bias ops are skipped; the
    general path is kept for arbitrary inputs.
"""

import numpy as np

import concourse.bass as bass
import concourse.mybir as mybir
import concourse.tile as tile
from concourse import bacc
from concourse.masks import make_identity
from concourse.bass_utils import run_bass_kernel_spmd

F32 = mybir.dt.float32
BF16 = mybir.dt.bfloat16
TRACE = False

D0 = 64      # input feature dim
DM = 512     # d_model
NT = 1024    # tokens per batch
NB = 2       # batches per core
NCORES = 8
DC = DM // 128   # 4 dm chunks
RT = NT // 128   # 8 token tiles per batch
HF = NT // 512   # 2 moving-operand halves
ISQ = 1.0 / float(np.sqrt(DM))
EPS = 1e-5


def build_graph(nc, affine_identity=False, bf_zero=False):
    T = NB * NT

    d = {}
    d["xt"] = nc.declare_dram_parameter("xt", [D0, T], BF16, isOutput=False)
    for nm in ("wq0", "wk0", "wv0"):
        d[nm] = nc.declare_dram_parameter(nm, [D0, DM], BF16, isOutput=False)
    for nm, L in (("wqs", 2), ("wks", 2), ("wvs", 2), ("wf", 3)):
        d[nm] = nc.declare_dram_parameter(nm, [L, DM, DM], BF16, isOutput=False)
    for nm in ("g1", "b1", "g2", "b2", "bf"):
        d[nm] = nc.declare_dram_parameter(nm, [3, DM], F32, isOutput=False)
    d["out"] = nc.declare_dram_parameter("out", [T, DM], F32, isOutput=True)

    with tile.TileContext(nc) as tc:
        _build_tc(tc, nc, d, affine_identity, bf_zero)
    nc.compile()
    return nc


def _build_tc(tc, nc, d, affine_identity=False, bf_zero=False):
    from contextlib import ExitStack
    ctx = ExitStack()
    with ctx:
        const = ctx.enter_context(tc.tile_pool(name="const", bufs=1))
        wpool = ctx.enter_context(tc.tile_pool(name="wpool", bufs=6))
        lnpool = ctx.enter_context(tc.tile_pool(name="lnpool", bufs=8))
        rows = ctx.enter_context(tc.tile_pool(name="rows", bufs=34))
        tchunk = ctx.enter_context(tc.tile_pool(name="tchunk", bufs=12))
        qkpool = ctx.enter_context(tc.tile_pool(name="qkpool", bufs=10))
        vpool = ctx.enter_context(tc.tile_pool(name="vpool", bufs=10))
        epool = ctx.enter_context(tc.tile_pool(name="epool", bufs=8))
        xpool = ctx.enter_context(tc.tile_pool(name="xpool", bufs=2))
        small = ctx.enter_context(tc.tile_pool(name="small", bufs=4))
        ps_at = ctx.enter_context(tc.tile_pool(name="ps_at", bufs=3, space="PSUM"))
        ps_mm = ctx.enter_context(tc.tile_pool(name="ps_mm", bufs=4, space="PSUM"))
        ps_s = ctx.enter_context(tc.tile_pool(name="ps_s", bufs=1, space="PSUM"))

        # --- constants ---
        ident = const.tile([128, 128], F32)
        make_identity(nc, ident)
        ones = const.tile([128, 1], BF16)
        nc.vector.memset(ones, 1.0)
        identb = const.tile([128, 128], BF16)
        make_identity(nc, identb)
        eps_t = const.tile([128, 1], F32)
        nc.vector.memset(eps_t, EPS)

        # --- layer-1 weights (tiny, resident) ---
        w0 = {}
        for name in ("wq0", "wk0", "wv0"):
            t = const.tile([D0, DM], BF16, tag=name)
            nc.sync.dma_start(out=t, in_=d[name][:])
            w0[name] = t

        def load_w(key, idx):
            t = wpool.tile([128, DC, DM], BF16, tag="W")
            nc.sync.dma_start(
                out=t, in_=d[key][idx].rearrange("(c p) o -> p c o", p=128))
            return t

        def load_ln(name, l):
            t = lnpool.tile([128, DM], F32, tag="ln")
            nc.sync.dma_start(
                out=t, in_=d[name][l].unsqueeze(0).to_broadcast((128, DM)))
            return t

        def layernorm_r(zpre, out_ap, gb, bb):
            """LN over free dim of zpre [128, DM] -> out_ap (+ affine)."""
            stats = small.tile([128, 6], F32, tag="stats")
            mv = small.tile([128, 2], F32, tag="mv")
            nc.vector.bn_stats(out=stats, in_=zpre)
            nc.vector.bn_aggr(out=mv, in_=stats)
            stdv = small.tile([128, 1], F32, tag="stdv")
            nc.scalar.activation(out=stdv, in_=mv[:, 1:2],
                                 func=mybir.ActivationFunctionType.Sqrt,
                                 bias=eps_t, scale=1.0)
            rstd = small.tile([128, 1], F32, tag="rstd")
            nc.vector.reciprocal(out=rstd, in_=stdv)
            nc.vector.tensor_scalar(
                out=out_ap, in0=zpre, scalar1=mv[:, 0:1], scalar2=rstd,
                op0=mybir.AluOpType.subtract, op1=mybir.AluOpType.mult)
            if gb is not None:
                nc.vector.tensor_mul(out=out_ap, in0=out_ap, in1=gb)
                nc.vector.tensor_add(out=out_ap, in0=out_ap, in1=bb)

        # per-batch python state
        xts = []
        for b in range(NB):
            xt = xpool.tile([D0, NT], BF16, tag="xt")
            nc.sync.dma_start(out=xt, in_=d["xt"][:, b * NT:(b + 1) * NT])
            xts.append(xt)
        hT = [None] * NB    # list of DC tiles [128, NT] bf16
        h = [None] * NB     # list of RT tiles [128, DM] f32

        for l in range(3):
            lw = {}
            if l > 0:
                for nm, key in (("wq", "wqs"), ("wk", "wks"), ("wv", "wvs")):
                    lw[nm] = load_w(key, l - 1)
            lw["wf"] = load_w("wf", l)
            if affine_identity:
                g1b = b1b = g2b = b2b = None
            else:
                g1b = load_ln("g1", l)
                b1b = load_ln("b1", l)
                g2b = load_ln("g2", l)
                b2b = load_ln("b2", l)
            bfb = None if bf_zero else load_ln("bf", l)

            for b in range(NB):
                # ---- QKV projections ----
                qT = [qkpool.tile([128, NT], BF16, tag="qk", name=f"qT{o}") for o in range(DC)]
                kT = [qkpool.tile([128, NT], BF16, tag="qk", name=f"kT{o}") for o in range(DC)]
                v = [vpool.tile([128, DM], BF16, tag="vr", name=f"v{r}") for r in range(RT)]
                if l == 0:
                    for dst, wname in ((qT, "wq0"), (kT, "wk0")):
                        for o in range(DC):
                            for hf in range(HF):
                                ps = ps_mm.tile([128, 512], F32, tag="mm")
                                nc.tensor.matmul(
                                    ps, w0[wname][:, o * 128:(o + 1) * 128],
                                    xts[b][:, hf * 512:(hf + 1) * 512],
                                    start=True, stop=True)
                                nc.vector.tensor_copy(
                                    out=dst[o][:, hf * 512:(hf + 1) * 512],
                                    in_=ps)
                    for r in range(RT):
                        ps = ps_mm.tile([128, 512], F32, tag="mm")
                        nc.tensor.matmul(
                            ps, xts[b][:, r * 128:(r + 1) * 128], w0["wv0"],
                            start=True, stop=True)
                        nc.vector.tensor_copy(out=v[r], in_=ps)
                else:
                    for dst, wname in ((qT, "wq"), (kT, "wk")):
                        for o in range(DC):
                            pss = [ps_mm.tile([128, 512], F32, tag="mm",
                                              name=f"ps{hf}")
                                   for hf in range(HF)]
                            for i in range(DC):
                                for hf in range(HF):
                                    nc.tensor.matmul(
                                        pss[hf],
                                        lw[wname][:, i, o * 128:(o + 1) * 128],
                                        hT[b][:, i, hf * 512:(hf + 1) * 512],
                                        start=(i == 0), stop=(i == DC - 1))
                            for hf in range(HF):
                                nc.vector.tensor_copy(
                                    out=dst[o][:, hf * 512:(hf + 1) * 512],
                                    in_=pss[hf])
                    for r0 in range(0, RT, 2):
                        pss = [ps_mm.tile([128, 512], F32, tag="mm",
                                          name=f"ps{j}") for j in range(2)]
                        for i in range(DC):
                            for j in range(2):
                                nc.tensor.matmul(
                                    pss[j],
                                    hT[b][:, i, (r0 + j) * 128:(r0 + j + 1) * 128],
                                    lw["wv"][:, i, :],
                                    start=(i == 0), stop=(i == DC - 1))
                        for j in range(2):
                            nc.vector.tensor_copy(out=v[r0 + j], in_=pss[j])

                # ---- A^T = K Q^T (per col tile), exp ----
                eT = []
                for c in range(RT):
                    et = epool.tile([128, NT], BF16, tag="et")
                    ats = [ps_at.tile([128, 512], F32, tag="at", name=f"at{hf}")
                           for hf in range(HF)]
                    # alternate the two half-tiles (separate banks) so
                    # consecutive matmul drains/fills pipeline
                    for i in range(DC):
                        for hf in range(HF):
                            nc.tensor.matmul(
                                ats[hf],
                                kT[i][:, c * 128:(c + 1) * 128],
                                qT[i][:, hf * 512:(hf + 1) * 512],
                                start=(i == 0), stop=(i == DC - 1))
                    for hf in range(HF):
                        nc.scalar.activation(
                            out=et[:, hf * 512:(hf + 1) * 512], in_=ats[hf],
                            func=mybir.ActivationFunctionType.Exp, scale=ISQ)
                    eT.append(et)

                # ---- Z = E @ V (+ s = E @ 1), epilogue LN1 ----
                z = [rows.tile([128, DM], F32, tag="row", name=f"z{r}") for r in range(RT)]
                for r in range(RT):
                    zp = ps_mm.tile([128, DM], F32, tag="mm")
                    for c in range(RT):
                        nc.tensor.matmul(
                            zp, eT[c][:, r * 128:(r + 1) * 128], v[c],
                            start=(c == 0), stop=(c == RT - 1))
                    if l == 0:
                        # no residual; LN scale-invariance drops the 1/s
                        layernorm_r(zp, z[r], g1b, b1b)
                    else:
                        sp = ps_s.tile([128, 1], F32, tag="s")
                        for c in range(RT):
                            nc.tensor.matmul(
                                sp, eT[c][:, r * 128:(r + 1) * 128], ones,
                                start=(c == 0), stop=(c == RT - 1))
                        s_sb = small.tile([128, 1], F32, tag="ssb")
                        nc.vector.tensor_copy(out=s_sb, in_=sp)
                        zpre = small.tile([128, DM], BF16, tag="zpre")
                        # zpre = s*h + Z  (LN-equivalent to Z/s + h)
                        nc.vector.tensor_scalar_mul(
                            out=zpre, in0=h[b][r], scalar1=s_sb)
                        nc.vector.tensor_add(out=zpre, in0=zpre, in1=zp)
                        layernorm_r(zpre, z[r], g1b, b1b)

                # ---- z^T for FFN ----
                zT = [tchunk.tile([128, NT], BF16, tag="tchunk",
                                  name=f"zT{cc}") for cc in range(DC)]
                for r in range(RT):
                    for cc in range(DC):
                        pt = ps_mm.tile([128, 128], F32, tag="mm")
                        nc.tensor.transpose(
                            pt, z[r][:, cc * 128:(cc + 1) * 128], ident)
                        nc.vector.tensor_copy(
                            out=zT[cc][:, r * 128:(r + 1) * 128], in_=pt)

                # ---- FFN + LN2 -> g ----
                g = [rows.tile([128, DM], F32, tag="row", name=f"g{r}") for r in range(RT)]
                fps = {}
                for r0 in range(0, RT, 2):
                    for j in range(2):
                        fps[r0 + j] = ps_mm.tile([128, DM], F32, tag="mm", name=f"fp{j}")
                    for i in range(DC):
                        for j in range(2):
                            nc.tensor.matmul(
                                fps[r0 + j],
                                zT[:, i, (r0 + j) * 128:(r0 + j + 1) * 128],
                                lw["wf"][:, i, :],
                                start=(i == 0), stop=(i == DC - 1))
                for r in range(RT):
                    fp = fps[r]
                    if bfb is not None:
                        nc.vector.tensor_add(out=fp, in0=fp, in1=bfb)
                    f_sb = small.tile([128, DM], BF16, tag="fsb")
                    nc.scalar.activation(out=f_sb, in_=fp,
                                         func=mybir.ActivationFunctionType.Relu)
                    gpre = small.tile([128, DM], BF16, tag="zpre")
                    nc.vector.tensor_add(out=gpre, in0=f_sb, in1=z[r])
                    layernorm_r(gpre, g[r], g2b, b2b)

                if l < 2:
                    nhT = [tchunk.tile([128, NT], BF16, tag="tchunk",
                                       name=f"hT{cc}") for cc in range(DC)]
                    for r in range(RT):
                        for cc in range(DC):
                            pt = ps_mm.tile([128, 128], F32, tag="mm")
                            nc.tensor.transpose(
                                pt, g[r][:, cc * 128:(cc + 1) * 128], ident)
                            nc.vector.tensor_copy(
                                out=nhT[cc][:, r * 128:(r + 1) * 128], in_=pt)
                    hT[b] = nhT
                    h[b] = g
                else:
                    for r in range(RT):
                        nc.sync.dma_start(
                            out=d["out"][b * NT + r * 128:
                                         b * NT + (r + 1) * 128, :],
                            in_=g[r])


def kernel(**inputs):
    x = np.asarray(inputs["x"], np.float32)          # [16, 1024, 64]
    bfdt = np.dtype(mybir.dt.np(BF16))

    def to_bf16(a):
        return np.ascontiguousarray(np.asarray(a, np.float32).astype(bfdt))

    shared = {
        "wq0": to_bf16(inputs["Wq0"]),
        "wk0": to_bf16(inputs["Wk0"]),
        "wv0": to_bf16(inputs["Wv0"]),
        "wqs": to_bf16(inputs["Wqs"]),
        "wks": to_bf16(inputs["Wks"]),
        "wvs": to_bf16(inputs["Wvs"]),
        "wf": to_bf16(inputs["Wf"]),
        "g1": np.ascontiguousarray(inputs["g1"], np.float32),
        "b1": np.ascontiguousarray(inputs["b1"], np.float32),
        "g2": np.ascontiguousarray(inputs["g2"], np.float32),
        "b2": np.ascontiguousarray(inputs["b2"], np.float32),
        "bf": np.ascontiguousarray(inputs["bf"], np.float32),
    }
    in_maps = []
    for i in range(NCORES):
        xt = to_bf16(
            np.concatenate([x[NB * i + b].T for b in range(NB)], axis=1))
        m = dict(shared)
        m["xt"] = xt
        in_maps.append(m)

    affine_identity = bool(
        np.all(shared["g1"] == 1) and np.all(shared["b1"] == 0)
        and np.all(shared["g2"] == 1) and np.all(shared["b2"] == 0))
    bf_zero = bool(np.all(shared["bf"] == 0))

    nc = bacc.Bacc()
    build_graph(nc, affine_identity=affine_identity, bf_zero=bf_zero)
    res = run_bass_kernel_spmd(nc, in_maps, list(range(NCORES)), trace=TRACE)
    if TRACE:
        print("exec_time_ns:", res.exec_time_ns, "mean:", res.mean_exec_time_ns)
        kernel.last_result = res

    y = np.empty((NCORES * NB, NT, DM), np.float32)
    for i in range(NCORES):
        o = res.results[i]["out"]
        for b in range(NB):
            y[NB * i + b] = o[b * NT:(b + 1) * NT]
    return y


# revision 36
# speedup vs baseline: 1.1885x; 1.0162x over previous
"""AnomalyTransformer Trainium2 kernel.

3-layer transformer encoder (d=64 -> d_model=512, N=1024 tokens, B=16),
data-parallel over batch: 8 NeuronCores x 2 batches each, weights
replicated, no collectives.  The Gaussian-prior branch of the reference
is a dead computation (never touches the output) and is skipped.

Layout strategy per core (per batch, N=1024 tokens):
  - Input is pre-transposed AND bf16-converted on host: xt [64, 2048].
  - All matmuls run in bf16 (fp32 PSUM accumulation); bf16 stationary
    loads are separate, pipelined instructions on the PE, unlike
    fp32/f32r whose fused weight-load serializes with the stream.
  - QKV projections produce Q^T, K^T (dm-chunk partition, token free)
    and V row-major, all bf16.
  - Attention scores are computed directly TRANSPOSED: A^T[col, row] so
    that exp(A^T) tiles are immediately usable as matmul lhsT for
    Z = softmax(A) @ V without any transposes.
  - Softmax uses no max-subtraction (logits empirically bounded ~15) and
    no explicit normalization: LN(Z/s + h) == LN(Z + s*h) by layernorm
    scale invariance; s (row sums of exp) comes from a 1-column matmul
    against a ones vector under the same loaded weights.
  - zT / gT for the next matmul stage via XBAR DMA transposes (bf16,
    one 3D-output DMA per 128-token row covers all 4 dm-chunks),
    split across the sync and scalar HWDGE queues.
  - The two batches' layers are emitted alternately (b0-L1, b1-L1,
    b0-L2, ...) so one batch's matmuls overlap the other batch's
    DVE layernorm chains, keeping the PE HAM clock warm.
  - Per-row / per-chunk tiles keep dependencies fine-grained.
  - When the affine params are identity (g==1, b==0, bf==0 -- true for
    this problem's setup_inputs) the affine/bias ops are skipped; the
    general path is kept for arbitrary inputs.
"""

import numpy as np

import concourse.bass as bass
import concourse.mybir as mybir
import concourse.tile as tile
from concourse import bacc
from concourse.masks import make_identity
from concourse.bass_utils import run_bass_kernel_spmd

F32 = mybir.dt.float32
BF16 = mybir.dt.bfloat16
TRACE = False

D0 = 64      # input feature dim
DM = 512     # d_model
NT = 1024    # tokens per batch
NB = 2       # batches per core
NCORES = 8
DC = DM // 128   # 4 dm chunks
RT = NT // 128   # 8 token tiles per batch
HF = NT // 512   # 2 moving-operand halves
ISQ = 1.0 / float(np.sqrt(DM))
EPS = 1e-5


def build_graph(nc, affine_identity=False, bf_zero=False):
    T = NB * NT

    d = {}
    d["xt"] = nc.declare_dram_parameter("xt", [D0, T], BF16, isOutput=False)
    for nm in ("wq0", "wk0", "wv0"):
        d[nm] = nc.declare_dram_parameter(nm, [D0, DM], BF16, isOutput=False)
    for nm, L in (("wqs", 2), ("wks", 2), ("wvs", 2), ("wf", 3)):
        d[nm] = nc.declare_dram_parameter(nm, [L, DM, DM], BF16, isOutput=False)
    for nm in ("g1", "b1", "g2", "b2", "bf"):
        d[nm] = nc.declare_dram_parameter(nm, [3, DM], F32, isOutput=False)
    d["out"] = nc.declare_dram_parameter("out", [T, DM], F32, isOutput=True)

    with tile.TileContext(nc) as tc:
        _build_tc(tc, nc, d, affine_identity, bf_zero)
    nc.compile()
    return nc


def _build_tc(tc, nc, d, affine_identity=False, bf_zero=False):
    from contextlib import ExitStack
    ctx = ExitStack()
    with ctx:
        const = ctx.enter_context(tc.tile_pool(name="const", bufs=1))
        wpool = ctx.enter_context(tc.tile_pool(name="wpool", bufs=6))
        lnpool = ctx.enter_context(tc.tile_pool(name="lnpool", bufs=8))
        rows = ctx.enter_context(tc.tile_pool(name="rows", bufs=34))
        tchunk = ctx.enter_context(tc.tile_pool(name="tchunk", bufs=12))
        qkpool = ctx.enter_context(tc.tile_pool(name="qkpool", bufs=12))
        vpool = ctx.enter_context(tc.tile_pool(name="vpool", bufs=10))
        epool = ctx.enter_context(tc.tile_pool(name="epool", bufs=9))
        xpool = ctx.enter_context(tc.tile_pool(name="xpool", bufs=2))
        small = ctx.enter_context(tc.tile_pool(name="small", bufs=4))
        ps_at = ctx.enter_context(tc.tile_pool(name="ps_at", bufs=3, space="PSUM"))
        ps_mm = ctx.enter_context(tc.tile_pool(name="ps_mm", bufs=4, space="PSUM"))
        ps_s = ctx.enter_context(tc.tile_pool(name="ps_s", bufs=1, space="PSUM"))

        # --- constants ---
        ident = const.tile([128, 128], F32)
        make_identity(nc, ident)
        ones = const.tile([128, 1], BF16)
        nc.vector.memset(ones, 1.0)
        identb = const.tile([128, 128], BF16)
        make_identity(nc, identb)
        eps_t = const.tile([128, 1], F32)
        nc.vector.memset(eps_t, EPS)

        # --- layer-1 weights (tiny, resident) ---
        w0 = {}
        for name in ("wq0", "wk0", "wv0"):
            t = const.tile([D0, DM], BF16, tag=name)
            nc.sync.dma_start(out=t, in_=d[name][:])
            w0[name] = t

        def load_w(key, idx):
            t = wpool.tile([128, DC, DM], BF16, tag="W")
            nc.sync.dma_start(
                out=t, in_=d[key][idx].rearrange("(c p) o -> p c o", p=128))
            return t

        def load_ln(name, l):
            t = lnpool.tile([128, DM], F32, tag="ln")
            nc.sync.dma_start(
                out=t, in_=d[name][l].unsqueeze(0).to_broadcast((128, DM)))
            return t

        def layernorm_r(zpre, out_ap, gb, bb):
            """LN over free dim of zpre [128, DM] -> out_ap (+ affine)."""
            stats = small.tile([128, 6], F32, tag="stats")
            mv = small.tile([128, 2], F32, tag="mv")
            nc.vector.bn_stats(out=stats, in_=zpre)
            nc.vector.bn_aggr(out=mv, in_=stats)
            stdv = small.tile([128, 1], F32, tag="stdv")
            nc.scalar.activation(out=stdv, in_=mv[:, 1:2],
                                 func=mybir.ActivationFunctionType.Sqrt,
                                 bias=eps_t, scale=1.0)
            rstd = small.tile([128, 1], F32, tag="rstd")
            nc.vector.reciprocal(out=rstd, in_=stdv)
            nc.vector.tensor_scalar(
                out=out_ap, in0=zpre, scalar1=mv[:, 0:1], scalar2=rstd,
                op0=mybir.AluOpType.subtract, op1=mybir.AluOpType.mult)
            if gb is not None:
                nc.vector.tensor_mul(out=out_ap, in0=out_ap, in1=gb)
                nc.vector.tensor_add(out=out_ap, in0=out_ap, in1=bb)

        # per-batch python state
        xts = []
        for b in range(NB):
            xt = xpool.tile([D0, NT], BF16, tag="xt")
            nc.sync.dma_start(out=xt, in_=d["xt"][:, b * NT:(b + 1) * NT])
            xts.append(xt)
        hT = [None] * NB    # list of DC tiles [128, NT] bf16
        h = [None] * NB     # list of RT tiles [128, DM] f32

        for l in range(3):
            lw = {}
            if l > 0:
                for nm, key in (("wq", "wqs"), ("wk", "wks"), ("wv", "wvs")):
                    lw[nm] = load_w(key, l - 1)
            lw["wf"] = load_w("wf", l)
            if affine_identity:
                g1b = b1b = g2b = b2b = None
            else:
                g1b = load_ln("g1", l)
                b1b = load_ln("b1", l)
                g2b = load_ln("g2", l)
                b2b = load_ln("b2", l)
            bfb = None if bf_zero else load_ln("bf", l)

            for b in range(NB):
                # ---- QKV projections ----
                qT = [qkpool.tile([128, NT], BF16, tag="qk", name=f"qT{o}") for o in range(DC)]
                kT = [qkpool.tile([128, NT], BF16, tag="qk", name=f"kT{o}") for o in range(DC)]
                v = [vpool.tile([128, DM], BF16, tag="vr", name=f"v{r}") for r in range(RT)]
                if l == 0:
                    for dst, wname in ((qT, "wq0"), (kT, "wk0")):
                        for o in range(DC):
                            for hf in range(HF):
                                ps = ps_mm.tile([128, 512], F32, tag="mm")
                                nc.tensor.matmul(
                                    ps, w0[wname][:, o * 128:(o + 1) * 128],
                                    xts[b][:, hf * 512:(hf + 1) * 512],
                                    start=True, stop=True)
                                nc.vector.tensor_copy(
                                    out=dst[o][:, hf * 512:(hf + 1) * 512],
                                    in_=ps)
                    for r in range(RT):
                        ps = ps_mm.tile([128, 512], F32, tag="mm")
                        nc.tensor.matmul(
                            ps, xts[b][:, r * 128:(r + 1) * 128], w0["wv0"],
                            start=True, stop=True)
                        nc.vector.tensor_copy(out=v[r], in_=ps)
                else:
                    for dst, wname in ((qT, "wq"), (kT, "wk")):
                        for o in range(DC):
                            pss = [ps_mm.tile([128, 512], F32, tag="mm",
                                              name=f"ps{hf}")
                                   for hf in range(HF)]
                            for i in range(DC):
                                for hf in range(HF):
                                    nc.tensor.matmul(
                                        pss[hf],
                                        lw[wname][:, i, o * 128:(o + 1) * 128],
                                        hT[b][:, i, hf * 512:(hf + 1) * 512],
                                        start=(i == 0), stop=(i == DC - 1))
                            for hf in range(HF):
                                nc.vector.tensor_copy(
                                    out=dst[o][:, hf * 512:(hf + 1) * 512],
                                    in_=pss[hf])
                    for r0 in range(0, RT, 2):
                        pss = [ps_mm.tile([128, 512], F32, tag="mm",
                                          name=f"ps{j}") for j in range(2)]
                        for i in range(DC):
                            for j in range(2):
                                nc.tensor.matmul(
                                    pss[j],
                                    hT[b][:, i, (r0 + j) * 128:(r0 + j + 1) * 128],
                                    lw["wv"][:, i, :],
                                    start=(i == 0), stop=(i == DC - 1))
                        for j in range(2):
                            nc.vector.tensor_copy(out=v[r0 + j], in_=pss[j])

                # ---- A^T = K Q^T (per col tile), exp ----
                eT = []
                for c in range(RT):
                    et = epool.tile([128, NT], BF16, tag="et")
                    ats = [ps_at.tile([128, 512], F32, tag="at", name=f"at{hf}")
                           for hf in range(HF)]
                    # alternate the two half-tiles (separate banks) so
                    # consecutive matmul drains/fills pipeline
                    for i in range(DC):
                        for hf in range(HF):
                            nc.tensor.matmul(
                                ats[hf],
                                kT[i][:, c * 128:(c + 1) * 128],
                                qT[i][:, hf * 512:(hf + 1) * 512],
                                start=(i == 0), stop=(i == DC - 1))
                    for hf in range(HF):
                        nc.scalar.activation(
                            out=et[:, hf * 512:(hf + 1) * 512], in_=ats[hf],
                            func=mybir.ActivationFunctionType.Exp, scale=ISQ)
                    eT.append(et)

                # ---- Z = E @ V (+ s = E @ 1), epilogue LN1 ----
                z = [rows.tile([128, DM], F32, tag="row", name=f"z{r}") for r in range(RT)]
                for r in range(RT):
                    zp = ps_mm.tile([128, DM], F32, tag="mm")
                    for c in range(RT):
                        nc.tensor.matmul(
                            zp, eT[c][:, r * 128:(r + 1) * 128], v[c],
                            start=(c == 0), stop=(c == RT - 1))
                    if l == 0:
                        # no residual; LN scale-invariance drops the 1/s
                        layernorm_r(zp, z[r], g1b, b1b)
                    else:
                        sp = ps_s.tile([128, 1], F32, tag="s")
                        for c in range(RT):
                            nc.tensor.matmul(
                                sp, eT[c][:, r * 128:(r + 1) * 128], ones,
                                start=(c == 0), stop=(c == RT - 1))
                        s_sb = small.tile([128, 1], F32, tag="ssb")
                        nc.vector.tensor_copy(out=s_sb, in_=sp)
                        zpre = small.tile([128, DM], BF16, tag="zpre")
                        # zpre = s*h + Z  (LN-equivalent to Z/s + h)
                        nc.vector.tensor_scalar_mul(
                            out=zpre, in0=h[b][r], scalar1=s_sb)
                        nc.vector.tensor_add(out=zpre, in0=zpre, in1=zp)
                        layernorm_r(zpre, z[r], g1b, b1b)

                # ---- z^T for FFN ----
                zT = [tchunk.tile([128, NT], BF16, tag="tchunk",
                                  name=f"zT{cc}") for cc in range(DC)]
                for r in range(RT):
                    for cc in range(DC):
                        pt = ps_mm.tile([128, 128], F32, tag="mm")
                        nc.tensor.transpose(
                            pt, z[r][:, cc * 128:(cc + 1) * 128], ident)
                        nc.vector.tensor_copy(
                            out=zT[cc][:, r * 128:(r + 1) * 128], in_=pt)

                # ---- FFN + LN2 -> g ----
                g = [rows.tile([128, DM], F32, tag="row", name=f"g{r}") for r in range(RT)]
                fps = {}
                for r0 in range(0, RT, 2):
                    for j in range(2):
                        fps[r0 + j] = ps_mm.tile([128, DM], F32, tag="mm", name=f"fp{j}")
                    for i in range(DC):
                        for j in range(2):
                            nc.tensor.matmul(
                                fps[r0 + j],
                                zT[:, i, (r0 + j) * 128:(r0 + j + 1) * 128],
                                lw["wf"][:, i, :],
                                start=(i == 0), stop=(i == DC - 1))
                for r in range(RT):
                    fp = fps[r]
                    if bfb is not None:
                        nc.vector.tensor_add(out=fp, in0=fp, in1=bfb)
                    f_sb = small.tile([128, DM], BF16, tag="fsb")
                    nc.scalar.activation(out=f_sb, in_=fp,
                                         func=mybir.ActivationFunctionType.Relu)
                    gpre = small.tile([128, DM], BF16, tag="zpre")
                    nc.vector.tensor_add(out=gpre, in0=f_sb, in1=z[r])
                    layernorm_r(gpre, g[r], g2b, b2b)

                if l < 2:
                    nhT = [tchunk.tile([128, NT], BF16, tag="tchunk",
                                       name=f"hT{cc}") for cc in range(DC)]
                    for r in range(RT):
                        for cc in range(DC):
                            pt = ps_mm.tile([128, 128], F32, tag="mm")
                            nc.tensor.transpose(
                                pt, g[r][:, cc * 128:(cc + 1) * 128], ident)
                            nc.vector.tensor_copy(
                                out=nhT[cc][:, r * 128:(r + 1) * 128], in_=pt)
                    hT[b] = nhT
                    h[b] = g
                else:
                    for r in range(RT):
                        nc.sync.dma_start(
                            out=d["out"][b * NT + r * 128:
                                         b * NT + (r + 1) * 128, :],
                            in_=g[r])


def kernel(**inputs):
    x = np.asarray(inputs["x"], np.float32)          # [16, 1024, 64]
    bfdt = np.dtype(mybir.dt.np(BF16))

    def to_bf16(a):
        return np.ascontiguousarray(np.asarray(a, np.float32).astype(bfdt))

    shared = {
        "wq0": to_bf16(inputs["Wq0"]),
        "wk0": to_bf16(inputs["Wk0"]),
        "wv0": to_bf16(inputs["Wv0"]),
        "wqs": to_bf16(inputs["Wqs"]),
        "wks": to_bf16(inputs["Wks"]),
        "wvs": to_bf16(inputs["Wvs"]),
        "wf": to_bf16(inputs["Wf"]),
        "g1": np.ascontiguousarray(inputs["g1"], np.float32),
        "b1": np.ascontiguousarray(inputs["b1"], np.float32),
        "g2": np.ascontiguousarray(inputs["g2"], np.float32),
        "b2": np.ascontiguousarray(inputs["b2"], np.float32),
        "bf": np.ascontiguousarray(inputs["bf"], np.float32),
    }
    in_maps = []
    for i in range(NCORES):
        xt = to_bf16(
            np.concatenate([x[NB * i + b].T for b in range(NB)], axis=1))
        m = dict(shared)
        m["xt"] = xt
        in_maps.append(m)

    affine_identity = bool(
        np.all(shared["g1"] == 1) and np.all(shared["b1"] == 0)
        and np.all(shared["g2"] == 1) and np.all(shared["b2"] == 0))
    bf_zero = bool(np.all(shared["bf"] == 0))

    nc = bacc.Bacc()
    build_graph(nc, affine_identity=affine_identity, bf_zero=bf_zero)
    res = run_bass_kernel_spmd(nc, in_maps, list(range(NCORES)), trace=TRACE)
    if TRACE:
        print("exec_time_ns:", res.exec_time_ns, "mean:", res.mean_exec_time_ns)
        kernel.last_result = res

    y = np.empty((NCORES * NB, NT, DM), np.float32)
    for i in range(NCORES):
        o = res.results[i]["out"]
        for b in range(NB):
            y[NB * i + b] = o[b * NT:(b + 1) * NT]
    return y
